# revision 52
# baseline (speedup 1.0000x reference)
"""Llama GQA chunk-attention layer on 8 Trainium2 NeuronCores.

Tensor-parallel over heads: core c computes Q heads [4c, 4c+4), KV head c,
and the partial output (attn_out_c @ Wo[rows of c's heads]); the host sums
the 8 partials (bf16 partials, f64 host accumulate).

Shapes (hardcoded): B=2, S=2048, HIDDEN=4096, 32 Q heads, 8 KV heads, D=128.
"""

import math
from contextlib import ExitStack

import ml_dtypes
import numpy as np

import concourse.bass as bass
import concourse.mybir as mybir
import concourse.tile as tile
from concourse.bass_utils import run_bass_kernel_spmd

# Problem constants
B, S, HID = 2, 2048, 4096
T = B * S                  # 4096 tokens
NH, NKV, D = 32, 8, 128
NCORES = 8
NH_C = NH // NCORES        # 4 q heads per core
DQ_C = NH_C * D            # 512
ROPE_BASE = 10000.0
SCALE = D ** -0.5

P = 128                    # partitions
TOKBLK = 256               # X^T chunk width for projections
NTOKBLK = T // TOKBLK      # 16
HO = HID // P              # 32 hidden 128-tiles
NTT = T // P               # 32 token 128-tiles
QBLK = 512                 # attention q-block width
NQB = S // QBLK            # 4 q-blocks per batch
KT_PER_B = S // P          # 16 k-tiles per batch

BF16 = mybir.dt.bfloat16
F32 = mybir.dt.float32
F8E5 = mybir.dt.float8e5


def build_bass():
    nc = bass.Bass()

    xt = nc.dram_tensor("xt", [HID, T], BF16, kind="ExternalInput")
    wq = nc.dram_tensor("wq", [HID, DQ_C], BF16, kind="ExternalInput")
    wk = nc.dram_tensor("wk", [HID, D], BF16, kind="ExternalInput")
    wv = nc.dram_tensor("wv", [HID, D], BF16, kind="ExternalInput")
    wo = nc.dram_tensor("wo", [DQ_C, HID], BF16, kind="ExternalInput")
    cos = nc.dram_tensor("cos", [D // 2, T], BF16, kind="ExternalInput")
    sin = nc.dram_tensor("sin", [D // 2, T], BF16, kind="ExternalInput")
    maskv = nc.dram_tensor("maskv", [P, P], BF16, kind="ExternalInput")
    ones = nc.dram_tensor("ones", [P, P], BF16, kind="ExternalInput")
    ones8 = nc.dram_tensor("ones8", [P, 2 * P], F8E5, kind="ExternalInput")
    out = nc.dram_tensor("out", [T, HID], BF16, kind="ExternalOutput")

    xt_r = xt.rearrange("(ho p) t -> p ho t", p=P)
    wq_r = wq.rearrange("(ho p) m -> p ho m", p=P)

    with tile.TileContext(nc) as tc, ExitStack() as ctx:
        singles = ctx.enter_context(tc.tile_pool(name="singles", bufs=1))
        xpool = ctx.enter_context(tc.tile_pool(name="xpool", bufs=3))
        tmps = ctx.enter_context(tc.tile_pool(name="tmps", bufs=1))
        ptpool = ctx.enter_context(tc.tile_pool(name="ptpool", bufs=5))
        p8pool = ctx.enter_context(tc.tile_pool(name="p8pool", bufs=2))
        rpool = ctx.enter_context(tc.tile_pool(name="rpool", bufs=1))
        opool = ctx.enter_context(tc.tile_pool(name="opool", bufs=3))

        # ---- resident SBUF tensors ----
        wq_sb = singles.tile([P, HO, DQ_C], BF16, tag="bigw")  # shared slot with wo
        wk_sb = singles.tile([P, HO, D], BF16, tag="wk")
        wv_sb = singles.tile([P, HO, D], BF16, tag="wv")
        qT_sb = singles.tile([P, NH_C, T], BF16, tag="qT")
        kT_sb = singles.tile([P, T], BF16, tag="kT")
        v_sb = singles.tile([P, NTT, D], BF16, tag="v")
        aT_sb = singles.tile([P, NH_C, T], BF16, tag="aT")     # attn_out^T
        # cos on partitions [0:64), sin on [64:128) — halves the SBUF
        # free-size reservation vs two 64-partition tiles
        cs_sb = singles.tile([P, T], BF16, tag="cossin")
        cos_sb = cs_sb[0:D // 2, :]
        sin_sb = cs_sb[D // 2:P, :]
        maskv_sb = singles.tile([P, P], BF16, tag="maskv")
        ones_sb = singles.tile([P, P], BF16, tag="ones")
        ones8_sb = singles.tile([P, 2, P], F8E5, tag="ones8")
        scratch = singles.tile([1, 4], F32, tag="scratch")

        # Startup-critical DMA order (SP queue processes in program order):
        # first wk quarter -> first xt0 half -> rest of wk -> rest of xt0 ->
        # per-head wq chunks -> cos/sin -> wv -> small constants. The first
        # K matmul only needs wk[:,0,:] and xt0[:,0,:], so it starts ~6us in.
        wk_r = wk.rearrange("(ho p) m -> p ho m", p=P)

        def load_xt(tb_i, parts=2, xt_sb=None):
            t0 = tb_i * TOKBLK
            if xt_sb is None:
                xt_sb = xpool.tile([P, HO, TOKBLK], BF16, tag="xt")
            step = HO // parts
            for i in range(parts):
                nc.sync.dma_start(
                    xt_sb[:, i * step:(i + 1) * step, :],
                    xt_r[:, i * step:(i + 1) * step, t0:t0 + TOKBLK])
            return xt_sb

        # Fine-grained startup interleave: the first K matmuls need only the
        # leading wk/xt0 quarters; later quarters stream in behind them.
        xt0_sb = xpool.tile([P, HO, TOKBLK], BF16, tag="xt")
        e = HO // 8
        for i in range(8):
            nc.sync.dma_start(wk_sb[:, i * e:(i + 1) * e, :],
                              wk_r[:, i * e:(i + 1) * e, :])
            nc.sync.dma_start(xt0_sb[:, i * e:(i + 1) * e, :],
                              xt_r[:, i * e:(i + 1) * e, 0:TOKBLK])
        nc.sync.dma_start(wq_sb[:, :, 0:P], wq_r[:, :, 0:P])
        nc.sync.dma_start(cs_sb[0:D // 2, :], cos[:, :])
        nc.sync.dma_start(cs_sb[D // 2:P, :], sin[:, :])
        nc.sync.dma_start(wq_sb[:, :, P:2 * P], wq_r[:, :, P:2 * P])
        nc.sync.dma_start(wq_sb[:, :, 2 * P:3 * P], wq_r[:, :, 2 * P:3 * P])
        nc.sync.dma_start(wq_sb[:, :, 3 * P:4 * P], wq_r[:, :, 3 * P:4 * P])
        nc.sync.dma_start(wv_sb, wv.rearrange("(ho p) m -> p ho m", p=P))
        xt1_sb = load_xt(1, parts=2)
        nc.sync.dma_start(maskv_sb, maskv[:, :])
        nc.sync.dma_start(ones_sb, ones[:, :])
        nc.sync.dma_start(ones8_sb, ones8.rearrange("p (k m) -> p k m", k=2))

        def rope(dst_lo, dst_hi, src_psum, t0):
            # dst = [src_lo*cos - src_hi*sin ; src_hi*cos + src_lo*sin]
            c_sl = cos_sb[:, t0:t0 + TOKBLK]
            s_sl = sin_sb[:, t0:t0 + TOKBLK]
            h = D // 2
            # 1-element copy absorbs the PE wait so the tensor_tensor ops
            # below carry <=1 sync wait (TT ISA slot limit).
            nc.vector.tensor_copy(scratch[0:1, 0:1], src_psum[0:1, 0:1])
            ta = tmps.tile([h, TOKBLK], F32, tag="ta")
            tb = tmps.tile([h, TOKBLK], F32, tag="tb")
            nc.vector.tensor_mul(ta, src_psum[0:h, :], c_sl)
            nc.vector.tensor_mul(tb, src_psum[h:P, :], s_sl)
            nc.vector.tensor_sub(dst_lo, ta, tb)
            ta2 = tmps.tile([h, TOKBLK], F32, tag="ta")
            tb2 = tmps.tile([h, TOKBLK], F32, tag="tb")
            nc.vector.tensor_mul(ta2, src_psum[h:P, :], c_sl)
            nc.vector.tensor_mul(tb2, src_psum[0:h, :], s_sl)
            nc.vector.tensor_add(dst_hi, ta2, tb2)

        with tc.tile_pool(name="ps_p1", bufs=6, space="PSUM") as ps_st:

            # ---- phase 1: projections + RoPE ----
            for tb_i in range(NTOKBLK):
                t0 = tb_i * TOKBLK
                if tb_i == 0:
                    xt_sb = xt0_sb
                elif tb_i == 1:
                    xt_sb = xt1_sb
                else:
                    xt_sb = load_xt(tb_i)

                # K^T [dk, tok]; rope; -> kT_sb  (first: wk lands first)
                psk = ps_st.tile([P, TOKBLK], F32, tag="st")
                for ho in range(HO):
                    nc.tensor.matmul(
                        psk, lhsT=wk_sb[:, ho, :], rhs=xt_sb[:, ho, :],
                        start=(ho == 0), stop=(ho == HO - 1),
                    )
                rope(kT_sb[0:D // 2, t0:t0 + TOKBLK],
                     kT_sb[D // 2:P, t0:t0 + TOKBLK], psk, t0)

                # Q^T [dq, tok] per head tile; rope; -> qT_sb
                for hq in range(NH_C):
                    psq = ps_st.tile([P, TOKBLK], F32, tag="st")
                    for ho in range(HO):
                        nc.tensor.matmul(
                            psq,
                            lhsT=wq_sb[:, ho, hq * P:(hq + 1) * P],
                            rhs=xt_sb[:, ho, :],
                            start=(ho == 0), stop=(ho == HO - 1),
                        )
                    rope(qT_sb[0:D // 2, hq, t0:t0 + TOKBLK],
                         qT_sb[D // 2:P, hq, t0:t0 + TOKBLK], psq, t0)

                # V [tok, dv] -> v_sb
                for tt in range(TOKBLK // P):
                    g = (t0 // P) + tt
                    psv = ps_st.tile([P, D], F32, tag="st")
                    for ho in range(HO):
                        nc.tensor.matmul(
                            psv,
                            lhsT=xt_sb[:, ho, tt * P:(tt + 1) * P],
                            rhs=wv_sb[:, ho, :],
                            start=(ho == 0), stop=(ho == HO - 1),
                        )
                    nc.any.tensor_copy(v_sb[:, g, :], psv)

        # ---- phases 2+3 merged ----
        # Attention is Act-bound (exp); the output projection is PE-bound.
        # Interleaving per (batch, q-block) group fills the PE slack of one
        # with the other; attention runs one group ahead so aT is ready.
        wo_sb = singles.tile([P, NH_C, HID], BF16, tag="bigw")
        nc.sync.dma_start(wo_sb, wo.rearrange("(h p) n -> p h n", p=P))

        with tc.tile_pool(name="ps_st", bufs=3, space="PSUM") as ps_st, \
             tc.tile_pool(name="ps_acc", bufs=2, space="PSUM") as ps_acc, \
             tc.tile_pool(name="ps_sum", bufs=1, space="PSUM") as ps_sum, \
             tc.tile_pool(name="ps_o", bufs=2, space="PSUM") as ps_o:

            def attn_unit(b, hq, qb):
                q0 = b * S + qb * QBLK
                nkt = (qb + 1) * (QBLK // P)
                ps_av = ps_acc.tile([P, QBLK], F32, tag="acc")
                ps_rs = ps_sum.tile([P, QBLK], F32, tag="sum")
                pr8 = None
                for kt in range(nkt):
                    gk = b * KT_PER_B + kt
                    j = kt - qb * (QBLK // P)
                    c0 = j * P if j >= 0 else 0
                    pst = ps_st.tile([P, QBLK], F32, tag="st")
                    nc.tensor.matmul(
                        pst[:, c0:],
                        lhsT=kT_sb[:, gk * P:(gk + 1) * P],
                        rhs=qT_sb[:, hq, q0 + c0:q0 + QBLK],
                        start=True, stop=True,
                    )
                    pT = ptpool.tile([P, QBLK], BF16, tag="pt")
                    nc.scalar.activation(
                        pT[:, c0:], pst[:, c0:],
                        mybir.ActivationFunctionType.Exp)
                    if j >= 0:
                        nc.vector.tensor_mul(
                            pT[:, c0:c0 + P], pT[:, c0:c0 + P], maskv_sb)
                        # diag tiles: bf16 rowsum on the live suffix; the
                        # all-ones stationary replicates the k-rowsum to
                        # every PSUM partition (no partition-broadcast)
                        nc.tensor.matmul(
                            ps_rs[:, c0:], lhsT=ones_sb[:, :],
                            rhs=pT[:, c0:],
                            start=(kt == 0), stop=(kt == nkt - 1),
                            skip_group_check=True,
                        )
                    else:
                        # non-diag tiles (even count, kt ascending): stage
                        # fp8e5 copies and fold each PAIR's rowsum into one
                        # DoubleRow matmul (256-deep contraction, 2x rate).
                        # e5m2 covers exp's range unscaled; its quantization
                        # only touches the denominators of long rows.
                        if pr8 is None:
                            pr8 = p8pool.tile([P, 2, QBLK], F8E5, tag="p8")
                        nc.vector.tensor_copy(pr8[:, kt % 2, :], pT)
                        if kt % 2 == 1:
                            nc.tensor.matmul(
                                ps_rs, lhsT=ones8_sb[:, :, :],
                                rhs=pr8[:, :, :],
                                start=(kt == 1), stop=False,
                                perf_mode=mybir.MatmulPerfMode.DoubleRow,
                                skip_group_check=True,
                            )
                            pr8 = None
                    nc.tensor.matmul(
                        ps_av[:, c0:], lhsT=v_sb[:, gk, :],
                        rhs=pT[:, c0:],
                        start=(kt == 0), stop=(kt == nkt - 1),
                        skip_group_check=True,
                    )
                r = rpool.tile([P, QBLK], F32, tag="r")
                nc.vector.reciprocal(r, ps_rs)
                nc.vector.tensor_mul(aT_sb[:, hq, q0:q0 + QBLK], ps_av, r)

            def p3_tt(tt, last_group):
                H2 = HID // 2
                NHB2 = HID // QBLK // 2
                for half in range(2):
                    o_t = opool.tile([P, H2], BF16, tag="ot")
                    for hbi in range(NHB2):
                        hb = half * NHB2 + hbi
                        pso = ps_o.tile([P, QBLK], F32, tag="o")
                        for hq in range(NH_C):
                            nc.tensor.matmul(
                                pso,
                                lhsT=aT_sb[:, hq, tt * P:(tt + 1) * P],
                                rhs=wo_sb[:, hq, hb * QBLK:(hb + 1) * QBLK],
                                start=(hq == 0), stop=(hq == NH_C - 1),
                            )
                        dst = o_t[:, hbi * QBLK:(hbi + 1) * QBLK]
                        # keep Act free for exp while groups interleave;
                        # share with Act in the pure-projection tail
                        if last_group and hb % 2 == 1:
                            nc.scalar.copy(dst, pso)
                        else:
                            nc.vector.tensor_copy(dst, pso)
                    if tt == NTT - 1 and half == 1:
                        for hbi in range(NHB2):
                            nc.sync.dma_start(
                                out[tt * P:(tt + 1) * P,
                                    half * H2 + hbi * QBLK:
                                    half * H2 + (hbi + 1) * QBLK],
                                o_t[:, hbi * QBLK:(hbi + 1) * QBLK])
                    else:
                        nc.sync.dma_start(
                            out[tt * P:(tt + 1) * P,
                                half * H2:(half + 1) * H2], o_t)

            groups = [(b, qb) for b in range(B) for qb in range(NQB)]
            for gi, (b, qb) in enumerate(groups):
                for hq in range(NH_C):
                    attn_unit(b, hq, qb)
                if gi >= 1:
                    pb, pqb = groups[gi - 1]
                    t0_ = pb * (NTT // 2) + pqb * 4
                    for tt in range(t0_, t0_ + 4):
                        p3_tt(tt, False)
            lb, lqb = groups[-1]
            t0_ = lb * (NTT // 2) + lqb * 4
            for tt in range(t0_, t0_ + 4):
                p3_tt(tt, True)

    return nc


def _prep_inputs(hidden_states, position_ids, Wq, Wk, Wv, Wo):
    bf16 = ml_dtypes.bfloat16
    hs = np.asarray(hidden_states, np.float32).reshape(T, HID)
    xt = np.ascontiguousarray(hs.T).astype(bf16)

    pos = np.asarray(position_ids, np.int64).reshape(B, S)
    inv_freq = 1.0 / (ROPE_BASE ** (np.arange(0, D, 2, dtype=np.float32) / D))
    ang = pos.astype(np.float32)[:, :, None] * inv_freq[None, None, :]  # [B,S,64]
    cosT = np.ascontiguousarray(
        np.cos(ang).reshape(T, D // 2).T).astype(bf16)  # [64, T]
    sinT = np.ascontiguousarray(
        np.sin(ang).reshape(T, D // 2).T).astype(bf16)

    Wq = np.asarray(Wq, np.float32) * SCALE  # fold softmax scale into Q
    Wk = np.asarray(Wk, np.float32)
    Wv = np.asarray(Wv, np.float32)
    Wo = np.asarray(Wo, np.float32)

    # staircase mask for the diagonal 128-block: valid iff col >= row
    maskv = np.triu(np.ones((P, P), np.float32)).astype(bf16)
    ones = np.ones((P, P), bf16)
    ones8 = np.ones((P, 2 * P), ml_dtypes.float8_e5m2)

    shared = {"xt": xt, "cos": cosT, "sin": sinT, "maskv": maskv,
              "ones": ones, "ones8": ones8}
    in_maps = []
    for c in range(NCORES):
        m = dict(shared)
        m["wq"] = np.ascontiguousarray(
            Wq[:, c * DQ_C:(c + 1) * DQ_C]).astype(bf16)
        m["wk"] = np.ascontiguousarray(Wk[:, c * D:(c + 1) * D]).astype(bf16)
        m["wv"] = np.ascontiguousarray(Wv[:, c * D:(c + 1) * D]).astype(bf16)
        m["wo"] = np.ascontiguousarray(
            Wo[c * DQ_C:(c + 1) * DQ_C, :]).astype(bf16)
        in_maps.append(m)
    return in_maps


_LAST_EXEC_NS = None


def legalize_sync_waits(js: bytes) -> bytes:
    """This walrus accepts at most one embedded sync wait per instruction.
    Hoist extra waits (and extra updates) onto standalone EventSemaphore
    instructions inserted just before (waits) / after (updates) on the same
    engine stream."""
    import json
    d = json.loads(js)
    n_new = 0
    for fn in d["functions"]:
        for blk in fn["blocks"]:
            out = []
            for inst in blk["instructions"]:
                si = inst.get("sync_info")
                waits = (si or {}).get("on_wait") or []
                updates = (si or {}).get("on_update") or []
                if len(waits) > 1:
                    for w in waits[:-1]:
                        n_new += 1
                        out.append({
                            "debug": inst.get("debug", 0),
                            "engine": inst["engine"],
                            "ins": [], "outs": [],
                            "name": f"{inst['name']}-hw{n_new}",
                            "opcode": "EventSemaphore",
                            "sync_info": {"on_wait": [w], "on_update": []},
                        })
                    si["on_wait"] = [waits[-1]]
                out.append(inst)
                if len(updates) > 1:
                    for u in updates[1:]:
                        n_new += 1
                        out.append({
                            "debug": inst.get("debug", 0),
                            "engine": inst["engine"],
                            "ins": [], "outs": [],
                            "name": f"{inst['name']}-hu{n_new}",
                            "opcode": "EventSemaphore",
                            "sync_info": {"on_wait": [], "on_update": [u]},
                        })
                    si["on_update"] = [updates[0]]
            blk["instructions"] = out
    return json.dumps(d).encode()


def _run(in_maps, trace=False):
    global _LAST_EXEC_NS
    nc = build_bass()
    legalized = legalize_sync_waits(nc.to_json_bytes())
    nc.to_json_bytes = lambda: legalized
    res = run_bass_kernel_spmd(nc, in_maps, core_ids=list(range(NCORES)),
                               trace=trace)
    _LAST_EXEC_NS = res.exec_time_ns
    return res.results


def kernel(hidden_states, position_ids, Wq, Wk, Wv, Wo):
    import os
    in_maps = _prep_inputs(hidden_states, position_ids, Wq, Wk, Wv, Wo)
    trace = bool(int(os.environ.get("KERNEL_TRACE", "0")))
    results = _run(in_maps, trace=trace)
    total = np.zeros((T, HID), np.float64)
    for r in results:
        total += np.asarray(r["out"]).astype(np.float64)
    return total.astype(np.float32).reshape(B, S, HID)
